# revision 34
# baseline (speedup 1.0000x reference)
# Trainium2 Bass kernel for nn_MLPMessageAggregator.
#
# Computation (per node n, per message-width column m):
#   x[k] = msgs[n, k, m] * valid[n, k]        (front-zero-pad mask from counts)
#   h1 = relu(x @ W1 + b1)   (6 -> 3)
#   h2 = relu(h1 @ W2 + b2)  (3 -> 2)
#   out[n, m] = h2 @ W3 + b3 (2 -> 1)
# timestamps pass through unchanged.
#
# Strategy (pure data parallel over N across 8 NeuronCores):
#   The tiny per-(n,m) contractions run on the TensorEngine (in float32r,
#   full-rate fp32-width mode) by packing (node, feature) pairs onto the
#   SBUF partitions with host-built block-diagonal weight matrices.
#   Nodes are processed in groups of 42; the K=6 slot contraction is
#   split by slot parity (k = 2*k3 + kp) into two accumulating matmuls
#   so that the contraction spans (42 nodes x 3 slots) = 126 partitions
#   and every PSUM output starts at partition 0 (an fp32r requirement):
#     mm1 (x2, accumulating): lhsT [126, 128] block-diag W1[kp::2] -> h1
#         on 126 partitions (g:42, j:3)
#     mm2: contraction 126, lhsT [126, 84] block-diag W2 -> 84 partitions
#     mm3: contraction 84, lhsT [84, 42] block-diag W3 -> partition = node
#   Masking is a per-partition tensor_scalar multiply on GpSimd in the
#   (g,k3)-partition layout (mask constant along the free dim), which also
#   rounds fp32 -> fp32r.  DVE evacuates layer-1 PSUM (fused +b1/relu via
#   dual-op tensor_scalar, m halves bank-paired into one strided read) and
#   layer 2 (+b2/relu, double-buffered single-bank PSUM); ACT evacuates
#   layer 3 (+b3) and issues the paired output-store DMAs.  All engines
#   run explicitly semaphore-pipelined instruction streams (raw bass, no
#   Tile scheduler).  Cost-model estimate: ~584 us/core (vs ~440 us HBM
#   bandwidth roofline; plain fp32 matmuls would be PE-bound at ~990 us).
#
# Each core processes 8192 nodes as 66 supertiles of 126 nodes; the last
# supertile is shifted to end exactly at node 8192 (overlap region is
# written twice with identical values).

import numpy as np

def _ensure_import_path():
    try:
        import concourse.bass  # noqa: F401
    except ImportError:
        import sys
        for p in ("/opt/trn_rl_repo", "/root/.axon_site/_ro/trn_rl_repo"):
            if p not in sys.path:
                sys.path.append(p)

N_CORES = 8
K = 6
M = 688
MH = 344          # half of M; PSUM bank holds <= 512 fp32
G1 = 21           # nodes per mm1 matmul group
ST = 126          # nodes per supertile

_LAST_RESULTS = None   # test harness introspection
_LAST_NC = None        # last built program (for _bench)
_LAST_INMAPS = None    # last per-core input maps (for _bench)


def _build_program(n_nodes, starts):
    """Raw-bass pipeline with explicit semaphores (fp32r matmuls).

    Per supertile ti (126 nodes = 3 groups of 42): SP DMAs msgs into
    rhs[ti%3] with partition layout (g:42, k3:3) and column layout
    (grp:3, kp:2, m:688) where k = kp*3 + k3; GpSimd applies 6 masking
    multiplies (per grp, kp) that also round fp32 -> fp32r; PE runs, per
    group, 2 accumulating mm1 per m-half (contraction 126 = 42 nodes x 3
    slots, W1 split into k-halves), then mm2 (contraction 126) and mm3
    (contraction 84) -- every PSUM dst starts at partition 0 (fp32r
    requirement).  DVE evacuates ps1 (bank-paired strided read, +b1/relu)
    and ps2 (+b2/relu); ACT evacuates ps3 (+b3) and issues paired
    output-store DMAs right after the evacuations it depends on.
    """
    from contextlib import ExitStack
    import concourse.bass as bass
    import concourse.mybir as mybir

    f32 = mybir.dt.float32
    f32r = mybir.dt.float32r
    AF = mybir.ActivationFunctionType
    ALU = mybir.AluOpType

    nc = bass.Bass("TRN2", target_bir_lowering=False, debug=False)

    msgs_t = nc.declare_dram_parameter("msgs", [n_nodes, K, M], f32, isOutput=False)
    mask_t = nc.declare_dram_parameter("maskd", [126, 6 * len(starts)], f32,
                                       isOutput=False)
    w1a_t = nc.declare_dram_parameter("bdw1a", [126, 128], f32r, isOutput=False)
    w1b_t = nc.declare_dram_parameter("bdw1b", [126, 128], f32r, isOutput=False)
    w2_t = nc.declare_dram_parameter("bdw2", [126, 84], f32r, isOutput=False)
    w3_t = nc.declare_dram_parameter("bdw3", [84, 42], f32r, isOutput=False)
    b1_t = nc.declare_dram_parameter("b1v", [128, 1], f32, isOutput=False)
    b2_t = nc.declare_dram_parameter("b2v", [84, 1], f32, isOutput=False)
    b3_t = nc.declare_dram_parameter("b3v", [42, 1], f32, isOutput=False)
    out_t = nc.declare_dram_parameter("out", [n_nodes, M], f32, isOutput=True)

    T = len(starts)

    def msgs_src(ti):
        # dims (g:42, k3:3, grp:3, (kp m):1376); k = 2*k3 + kp: the two kp
        # slots are DRAM-adjacent, so (kp m) merges into 5504 B runs and
        # (g,k3) merges too -> one 3-D 2.08 MB DMA per supertile
        n0 = starts[ti]
        return msgs_t[n0:n0 + ST, :, :].rearrange(
            "(grp g) (k3 two) m -> g k3 grp (two m)", g=42, two=2)

    def out_dst(ti):
        n0 = starts[ti]
        # src ot iterates (p=g:42, col=(grp:3, m:688))
        return out_t[n0:n0 + ST, :].rearrange("(grp g) m -> g grp m", g=42)

    with ExitStack() as ctx:
        en = ctx.enter_context
        NB = 4          # rhs ring depth
        rhs = [en(nc.sbuf_tensor(f"rhs{i}", [126, 6 * M], f32)) for i in range(NB)]
        rhs_r = [en(nc.sbuf_tensor(f"rhsr{i}", [126, 6 * M], f32r))
                 for i in range(NB)]
        h1 = [en(nc.sbuf_tensor(f"h1_{i}", [126, 3 * M], f32r)) for i in range(2)]
        h2 = [en(nc.sbuf_tensor(f"h2_{i}", [84, 3 * M], f32r)) for i in range(2)]
        ot = [en(nc.sbuf_tensor(f"ot{i}", [42, 6 * M], f32)) for i in range(2)]
        w1a = en(nc.sbuf_tensor("w1a", [126, 128], f32r))
        w1b = en(nc.sbuf_tensor("w1b", [126, 128], f32r))
        w2s = en(nc.sbuf_tensor("w2s", [126, 84], f32r))
        w3s = en(nc.sbuf_tensor("w3s", [84, 42], f32r))
        b1s = en(nc.sbuf_tensor("b1s", [128, 1], f32))
        b2s = en(nc.sbuf_tensor("b2s", [84, 1], f32))
        b3s = en(nc.sbuf_tensor("b3s", [42, 1], f32))
        mtm = en(nc.sbuf_tensor("mtm", [126, 6 * T], f32))
        # PSUM: every tile pairs the two m-halves at bank stride
        ps1 = [en(nc.psum_tensor(f"ps1_{i}", [128, 2, 512], f32)) for i in range(2)]
        ps2 = [en(nc.psum_tensor(f"ps2_{i}", [128, 512], f32)) for i in range(2)]
        ps3 = [en(nc.psum_tensor(f"ps3_{i}", [128, 512], f32)) for i in range(2)]

        sCONST = en(nc.semaphore("sCONST"))
        sIN = [en(nc.semaphore(f"sIN{i}")) for i in range(NB)]
        sMASK = en(nc.semaphore("sMASK"))
        sMM1 = en(nc.semaphore("sMM1"))
        sMM2 = en(nc.semaphore("sMM2"))
        sMM3 = en(nc.semaphore("sMM3"))
        sEV1 = en(nc.semaphore("sEV1"))
        sEV2 = en(nc.semaphore("sEV2"))
        sEV3 = en(nc.semaphore("sEV3"))
        sOUT = [en(nc.semaphore(f"sOUT{i}")) for i in range(2)]

        const_pairs = [
            (w1a[:, :], w1a_t[:, :]), (w1b[:, :], w1b_t[:, :]),
            (w2s[:, :], w2_t[:, :]), (w3s[:, :], w3_t[:, :]),
            (b1s[:, :], b1_t[:, :]), (b2s[:, :], b2_t[:, :]),
            (b3s[:, :], b3_t[:, :]), (mtm[:, :], mask_t[:, :]),
        ]
        n_const = len(const_pairs)

        # Output pairs: two adjacent supertiles share one store DMA when
        # their node ranges are contiguous; otherwise per-supertile DMAs.
        n_pairs = (T + 1) // 2
        pair_members = [[u for u in (2 * P, 2 * P + 1) if u < T]
                        for P in range(n_pairs)]
        pair_double = [len(m) == 2 and starts[m[1]] == starts[m[0]] + ST
                       for m in pair_members]
        pair_ndma = [1 if d else len(m)
                     for m, d in zip(pair_members, pair_double)]
        # cumulative 16-incs on sOUT[P % 2] after each pair completes
        cum_out = {}
        tot_out = [0, 0]
        for P in range(n_pairs):
            tot_out[P % 2] += 16 * pair_ndma[P]
            cum_out[P] = tot_out[P % 2]

        with nc.Block() as block:

            @block.sync
            def _(sp):
                for dst, src_ap in const_pairs:
                    sp.dma_start(dst, src_ap).then_inc(sCONST, 16)
                for ti in range(T):
                    if ti >= NB:
                        sp.wait_ge(sMASK, ti - NB + 1)      # rhs buf free
                    sp.dma_start(rhs[ti % NB][:, :], msgs_src(ti)
                                 ).then_inc(sIN[ti % NB], 16)


            @block.gpsimd
            def _(gp):
                gp.wait_ge(sCONST, 16 * n_const)
                for ti in range(T):
                    if ti >= NB:
                        gp.wait_ge(sMM1, 3 * (ti - NB + 1))  # rhs_r buf free
                    gp.wait_ge(sIN[ti % NB], 16 * (ti // NB + 1))
                    for c in range(6):                       # c = kp*3 + grp
                        kp, grp = c // 3, c % 3
                        col = (grp * 2 + kp) * M
                        op = gp.tensor_scalar_mul(
                            rhs_r[ti % NB][:, col:col + M],
                            rhs[ti % NB][:, col:col + M],
                            mtm[:, ti * 6 + c:ti * 6 + c + 1])
                        if c == 5:
                            op.then_inc(sMASK, 1)

            @block.tensor
            def _(pe):
                pe.wait_ge(sCONST, 16 * n_const)

                def mm1(ti):
                    pe.wait_ge(sMASK, ti + 1)
                    for grp in range(3):
                        j = 3 * ti + grp
                        if j >= 2:
                            pe.wait_ge(sEV1, j - 1)         # ps1 buf free
                        b = j % 2
                        for h in range(2):
                            for kp in range(2):
                                col = (grp * 2 + kp) * M
                                op = nc.tensor.matmul(
                                    ps1[b][0:128, h, 0:MH],
                                    w1a[:, :] if kp == 0 else w1b[:, :],
                                    rhs_r[ti % NB][:, col + h * MH:
                                                   col + h * MH + MH],
                                    start=(kp == 0), stop=(kp == 1))
                        op.then_inc(sMM1, 1)

                def mm2(ti):
                    for grp in range(3):
                        pe.wait_ge(sEV1, 3 * ti + grp + 1)  # h1 grp cols ready
                        for h in range(2):
                            k = 6 * ti + 2 * grp + h
                            if k >= 2:
                                pe.wait_ge(sEV2, k - 1)     # ps2 buf free
                            nc.tensor.matmul(
                                ps2[k % 2][0:84, 0:MH], w2s[:, :],
                                h1[ti % 2][:, grp * M + h * MH:
                                           grp * M + h * MH + MH],
                                start=True, stop=True).then_inc(sMM2, 1)

                def mm3(ti):
                    for grp in range(3):
                        for h in range(2):
                            k = 6 * ti + 2 * grp + h
                            pe.wait_ge(sEV2, k + 1)         # h2 cols ready
                            if k >= 2:
                                pe.wait_ge(sEV3, k - 1)     # ps3 buf free
                            nc.tensor.matmul(
                                ps3[k % 2][0:42, 0:MH], w3s[0:84, :],
                                h2[ti % 2][0:84, grp * M + h * MH:
                                           grp * M + h * MH + MH],
                                start=True, stop=True).then_inc(sMM3, 1)

                mm1(0)
                for ti in range(T):
                    if ti + 1 < T:
                        mm1(ti + 1)
                    mm2(ti)
                    mm3(ti)

            @block.scalar
            def _(act):
                act.wait_ge(sCONST, 16 * n_const)

                def issue_pair(P):
                    b = P % 2
                    mem = pair_members[P]
                    # same-engine, but the DMA issues before the prior
                    # activation's writeback -- wait on its semaphore
                    act.wait_ge(sEV3, 6 * (mem[-1] + 1))
                    if pair_double[P]:
                        n0 = starts[mem[0]]
                        act.dma_start(
                            out_t[n0:n0 + 2 * ST, :].rearrange(
                                "(tq grp g) m -> g tq grp m", g=42, grp=3),
                            ot[b][0:42, :]).then_inc(sOUT[b], 16)
                    else:
                        for q, u in enumerate(mem):
                            act.dma_start(
                                out_dst(u),
                                ot[b][0:42, q * 3 * M:(q + 1) * 3 * M]
                            ).then_inc(sOUT[b], 16)

                def evac3(ti):
                    P = ti // 2
                    q = ti % 2
                    if P >= 2 and q == 0:
                        act.wait_ge(sOUT[P % 2], cum_out[P - 2])  # ot buf free
                    for grp in range(3):
                        for h in range(2):
                            k = 6 * ti + 2 * grp + h
                            act.wait_ge(sMM3, k + 1)
                            nc.scalar.activation(
                                ot[P % 2][0:42,
                                          (q * 3 + grp) * M + h * MH:
                                          (q * 3 + grp) * M + h * MH + MH],
                                ps3[k % 2][0:42, 0:MH],
                                AF.Identity, bias=b3s[0:42, 0:1], scale=1.0,
                            ).then_inc(sEV3, 1)

                for ti in range(T):
                    evac3(ti)
                    if ti % 2 == 1:
                        issue_pair(ti // 2)
                if T % 2 == 1:
                    issue_pair(n_pairs - 1)
                for b in range(2):
                    if tot_out[b]:
                        act.wait_ge(sOUT[b], tot_out[b])

            @block.vector
            def _(dve):
                dve.wait_ge(sCONST, 16 * n_const)

                def evac1(ti):
                    if ti >= 2:
                        dve.wait_ge(sMM2, 6 * (ti - 1))     # h1 buf free
                    for grp in range(3):
                        j = 3 * ti + grp
                        dve.wait_ge(sMM1, j + 1)
                        nc.vector.tensor_scalar(
                            h1[ti % 2][:, grp * M:(grp + 1) * M],
                            ps1[j % 2][0:126, :, 0:MH],
                            b1s[0:126, 0:1], 0.0,
                            op0=ALU.add, op1=ALU.max,
                        ).then_inc(sEV1, 1)

                def evac2(ti):
                    if ti >= 2:
                        dve.wait_ge(sMM3, 6 * (ti - 1))     # h2 buf free
                    for grp in range(3):
                        for h in range(2):
                            k = 6 * ti + 2 * grp + h
                            dve.wait_ge(sMM2, k + 1)
                            nc.vector.tensor_scalar(
                                h2[ti % 2][0:84, grp * M + h * MH:
                                           grp * M + h * MH + MH],
                                ps2[k % 2][0:84, 0:MH],
                                b2s[0:84, 0:1], 0.0,
                                op0=ALU.add, op1=ALU.max,
                            ).then_inc(sEV2, 1)

                evac1(0)
                for ti in range(T):
                    if ti + 1 < T:
                        evac1(ti + 1)
                    evac2(ti)

    return nc


def _host_prep(W1, b1, W2, b2, W3, b3, counts):
    W1 = np.asarray(W1, np.float32)
    W2 = np.asarray(W2, np.float32)
    W3 = np.asarray(W3, np.float32)
    b1 = np.asarray(b1, np.float32)
    b2 = np.asarray(b2, np.float32)
    b3 = np.asarray(b3, np.float32)
    counts = np.asarray(counts)

    mask = (np.arange(K)[None, :] >= (K - counts[:, None].astype(np.int64))
            ).astype(np.float32)                       # [N, K]

    bdw1a = np.zeros((126, 128), np.float32)
    bdw1b = np.zeros((126, 128), np.float32)
    for g in range(42):
        bdw1a[g * 3:(g + 1) * 3, g * 3:(g + 1) * 3] = W1[0::2, :]   # k even
        bdw1b[g * 3:(g + 1) * 3, g * 3:(g + 1) * 3] = W1[1::2, :]   # k odd
    bdw2 = np.zeros((126, 84), np.float32)
    for g in range(42):
        for i in range(3):
            bdw2[g * 3 + i, g * 2:g * 2 + 2] = W2[i, :]
    bdw3 = np.zeros((84, 42), np.float32)
    for g in range(42):
        bdw3[g * 2:g * 2 + 2, g] = W3[:, 0]
    b1v = np.zeros((128, 1), np.float32)
    b1v[0:126, 0] = np.tile(b1, 42)
    b2v = np.zeros((84, 1), np.float32)
    b2v[:, 0] = np.tile(b2, 42)
    b3v = np.full((42, 1), float(b3[0]), np.float32)
    return mask, bdw1a, bdw1b, bdw2, bdw3, b1v, b2v, b3v


def _starts_for(n_nodes):
    starts = list(range(0, n_nodes - ST + 1, ST))
    if starts[-1] + ST < n_nodes:
        starts.append(n_nodes - ST)
    return starts


def _mask_device_layout(mask, starts):
    # [126, 6*T]: maskd[g*3+k3, ti*6 + kp*3 + grp]
    #   = mask[starts[ti] + grp*42 + g, 2*k3 + kp]
    cols = []
    for n0 in starts:
        m4 = mask[n0:n0 + ST].reshape(3, 42, 3, 2)     # (grp, g, k3, kp)
        cols.append(m4.transpose(1, 2, 3, 0).reshape(126, 6))
    return np.ascontiguousarray(np.concatenate(cols, axis=1))


def kernel(msgs, counts, timestamps, W1, b1, W2, b2, W3, b3):
    global _LAST_RESULTS
    _ensure_import_path()
    from concourse.bass_utils import run_bass_kernel_spmd

    msgs = np.ascontiguousarray(np.asarray(msgs, np.float32))
    n_total = msgs.shape[0]
    n_per = n_total // N_CORES

    mask, bdw1a, bdw1b, bdw2, bdw3, b1v, b2v, b3v = _host_prep(
        W1, b1, W2, b2, W3, b3, counts)

    starts = _starts_for(n_per)
    nc = _build_program(n_per, starts)

    in_maps = []
    for c in range(N_CORES):
        sl = slice(c * n_per, (c + 1) * n_per)
        in_maps.append({
            "msgs": msgs[sl],
            "maskd": _mask_device_layout(mask[sl], starts),
            "bdw1a": bdw1a, "bdw1b": bdw1b, "bdw2": bdw2, "bdw3": bdw3,
            "b1v": b1v, "b2v": b2v, "b3v": b3v,
        })

    global _LAST_NC, _LAST_INMAPS
    _LAST_NC = nc
    _LAST_INMAPS = in_maps
    res = run_bass_kernel_spmd(nc, in_maps, list(range(N_CORES)))
    _LAST_RESULTS = res

    out = np.concatenate([res.results[c]["out"] for c in range(N_CORES)], axis=0)
    return out, np.asarray(timestamps)


def _bench(n_iters=20):
    """Time repeated executions of the last-built program on the 8 cores.

    Inputs are staged on device once; output buffers are created on device
    per iteration (they are donated to the kernel).  Returns (serial_times,
    pipelined_avg): per-call wall seconds with a sync per call, and the
    average per-call time when all iterations are enqueued back-to-back.
    """
    import time as _t
    import jax
    import jax.numpy as jnp
    from jax.sharding import Mesh, PartitionSpec, NamedSharding
    from jax.experimental.shard_map import shard_map
    from concourse import bass2jax
    import concourse.mybir as mybir

    nc, in_maps = _LAST_NC, _LAST_INMAPS
    assert nc is not None, "call kernel() first"
    bass2jax.install_neuronx_cc_hook()

    partition_name = (nc.partition_id_tensor.name
                      if nc.partition_id_tensor else None)
    in_names, out_names, out_avals = [], [], []
    for alloc in nc.m.functions[0].allocations:
        if not isinstance(alloc, mybir.MemoryLocationSet):
            continue
        name = alloc.memorylocations[0].name
        if alloc.kind == "ExternalInput":
            if name != partition_name:
                in_names.append(name)
        elif alloc.kind == "ExternalOutput":
            out_names.append(name)
            out_avals.append(jax.core.ShapedArray(
                tuple(alloc.tensor_shape), mybir.dt.np(alloc.dtype)))
    n_params = len(in_names)
    all_names = in_names + out_names
    if partition_name is not None:
        all_names = all_names + [partition_name]
    donate = tuple(range(n_params, n_params + len(out_names)))

    def _body(*args):
        operands = list(args)
        if partition_name is not None:
            operands.append(bass2jax.partition_id_tensor())
        outs = bass2jax._bass_exec_p.bind(
            *operands, out_avals=tuple(out_avals), in_names=tuple(all_names),
            out_names=tuple(out_names), lowering_input_output_aliases=(),
            sim_require_finite=True, sim_require_nnan=True, nc=nc)
        return tuple(outs)

    devices = jax.devices()[:N_CORES]
    mesh = Mesh(np.asarray(devices), ("core",))
    in_specs = (PartitionSpec("core"),) * (n_params + len(out_names))
    out_specs = (PartitionSpec("core"),) * len(out_names)
    fn = jax.jit(shard_map(_body, mesh=mesh, in_specs=in_specs,
                           out_specs=out_specs, check_rep=False),
                 donate_argnums=donate, keep_unused=True)
    shard = NamedSharding(mesh, PartitionSpec("core"))
    concat_in = [np.concatenate([np.asarray(in_maps[c][nm])
                                 for c in range(N_CORES)], axis=0)
                 for nm in in_names]
    in_dev = [jax.device_put(a, shard) for a in concat_in]

    zfn = jax.jit(
        lambda: tuple(jnp.zeros((N_CORES * a.shape[0], *a.shape[1:]), a.dtype)
                      for a in out_avals),
        out_shardings=tuple(shard for _ in out_avals))

    out = fn(*in_dev, *zfn())          # warmup / compile
    jax.block_until_ready(out)

    serial = []
    for _ in range(n_iters):
        zs = zfn()
        jax.block_until_ready(zs)
        t0 = _t.perf_counter()
        out = fn(*in_dev, *zs)
        jax.block_until_ready(out)
        serial.append(_t.perf_counter() - t0)

    zsets = [zfn() for _ in range(n_iters)]
    jax.block_until_ready(zsets)
    t0 = _t.perf_counter()
    outs = [fn(*in_dev, *zs) for zs in zsets]
    jax.block_until_ready(outs)
    piped = (_t.perf_counter() - t0) / n_iters
    return serial, piped
